# revision 24
# baseline (speedup 1.0000x reference)
"""BinaryConv (XNOR-style binary-weight 3x3 conv) on 8 Trainium2 NeuronCores.

Full-input contract: kernel(x=[32,256,56,56] f32, weight=[256,256,3,3] f32)
-> [32,256,56,56] f32.

Strategy: data-parallel over batch (4 images/core), weight replicated.
Per core, a 1-D Winograd F(4,3) decomposition along W cuts the tensor-engine
MAC count 2x vs direct convolution (6 transformed positions per 4 output
columns instead of 12 tap-MACs):

  y = A'^T [ (G' s) . (B^T d) ]   per output row, with the 3 kh taps and the
                                  2 ci chunks accumulated in PSUM.

All math on device; host marshalling is layout/dtype only: x ships bf16,
de-interleaved into the six B^T operand planes d0..d5 (stride-4 phases of the
zero-padded rows, flattened 58x15 with a garbage 15th column) so every DVE op
and matmul rhs reads a contiguous, 4B-aligned window.  The weight transform
G'·sign(w) uses the row-rescaled dyadic G' (rows x[1,3,3,12,12,1]) so U is
exact in bf16; the matching column scales 1/c_p fold into the per-channel
a=mean|w| (ACT |.|+accum passes) applied at PSUM eviction on ACT.  Each
(co,p) pair accumulates into a 2-bank PSUM pair-tile so one eviction drains
both output batches into a contiguous bf16 E plane.  The input transform B^T
and inverse transform A'^T run on DVE in bf16 tensor_tensor (2x mode) with
unary scales on DVE(4x)/ACT; image 0's transforms are interleaved across
both ci chunks plane-by-plane (with the U-build woven in) so the PE ramps
immediately, and the last image's inverse transform overlaps its second
co-chunk matmuls.  The output ships phase-major bf16, re-interleaved f32 on
host.
"""

import ml_dtypes
import numpy as np

import concourse.mybir as mybir
import concourse.tile as tile
from concourse import bacc
from concourse.bass_utils import run_bass_kernel_spmd

F32 = mybir.dt.float32
BF16 = mybir.dt.bfloat16
ALU = mybir.AluOpType

N_CORES = 8
B, C, H, W = 32, 256, 56, 56
O, KH, KW = 256, 3, 3
BP = B // N_CORES            # images per core
P = 128                      # partitions
NCI = C // P                 # input-channel chunks
NCO = O // P                 # output-channel chunks
NP = 6                       # winograd positions (F(4,3): m+r-1 = 6)
TX = 14                      # output tiles along W (4 cols each)
PR = H + 2                   # padded rows (h -1..56)
PW15 = 15                    # tile columns incl. garbage col 14
DPL = PR * PW15 + 2          # 872: d-plane stride (870 valid + 2 pad)
FLAT = PR * PW15             # 870: flat transform window
OUTF = H * PW15              # 840: output flat length per (co, img)
FB = OUTF // 2               # 420: psum free size (2 batches)
KIN = C * KH * KW            # 2304 per-filter fan-in


def _u_off(p: int, kh: int, ci: int, co: int) -> int:
    return (((p * KH + kh) * NCI + ci) * NCO + co) * P


def build(bp: int = BP):
    nc = bacc.Bacc(
        "TRN2",
        target_bir_lowering=False,
        debug=False,
        enable_asserts=False,
        num_devices=N_CORES,
        enable_partition_id=False,
    )
    x_d = nc.dram_tensor("x", [bp, C, NP, DPL], BF16, kind="ExternalInput")
    w_d = nc.dram_tensor("w", [O, KIN], BF16, kind="ExternalInput")
    wp_d = nc.dram_tensor("wp", [KH * KW, C, O], BF16, kind="ExternalInput")
    out_d = nc.dram_tensor("out", [bp, O, 4, OUTF], BF16,
                           kind="ExternalOutput")

    x = x_d.ap().rearrange("n (c p) s f -> n p c s f", p=P)
    w = w_d.ap()
    wp = wp_d.ap().rearrange("t (c p) o -> p c t o", p=P)
    out = out_d.ap()

    with tile.TileContext(nc) as tc:
        with (
            tc.tile_pool(name="const", bufs=1) as const_pool,
            tc.tile_pool(name="wstage", bufs=2) as wstage_pool,
            tc.tile_pool(name="xph", bufs=2) as xph_pool,
            tc.tile_pool(name="vt", bufs=4) as v_pool,
            tc.tile_pool(name="tmp", bufs=13) as tmp_pool,
            tc.tile_pool(name="ev", bufs=7) as e_pool,
            tc.tile_pool(name="yt", bufs=13) as y_pool,
            tc.tile_pool(name="psum", bufs=3, space="PSUM") as psum_pool,
            tc.tile_pool(name="warmps", bufs=1, space="PSUM") as warmps_pool,
        ):
            # ---- PE warmup: hold HAM at 2.4GHz through the input ramp ----
            warm_l = const_pool.tile([P, P], BF16)
            warm_r = const_pool.tile([P, 512], BF16)
            nc.gpsimd.memset(warm_l[:], 0.0)
            nc.gpsimd.memset(warm_r[:], 0.0)
            zbias = const_pool.tile([P, 1], F32)
            zscr = const_pool.tile([P, 1], F32)
            nc.gpsimd.memset(zbias[:], 0.0)
            warm_ps = warmps_pool.tile([P, 512], F32)
            for _ in range(14):
                nc.tensor.matmul(warm_ps[:], warm_l[:], warm_r[:],
                                 start=True, stop=True)
            # preload the Sign LUT on ACT before the weights arrive
            nc.scalar.sign(zscr[:], zbias[:], bias=zbias[:])

            # ---- critical-path input DMAs on the sync ring (FIFO) --------
            wpt = [const_pool.tile([P, KH * KW, O], BF16, name=f"wpt{ci}")
                   for ci in range(NCI)]
            wstage = wstage_pool.tile([P, KIN], BF16, name="ws")
            wstage2 = wstage_pool.tile([P, KIN], BF16, name="ws2")
            xts: list[list] = [[None] * NCI for _ in range(bp)]

            def emit_x_dma(n, ci):
                xt = xph_pool.tile([P, NP, DPL], BF16, name="xt")
                nc.sync.dma_start(xt[:], x[n, :, ci])
                xts[n][ci] = xt

            nc.sync.dma_start(wpt[0][:], wp[:, 0])
            emit_x_dma(0, 0)
            emit_x_dma(0, 1)
            nc.sync.dma_start(wpt[1][:], wp[:, 1])
            nc.sync.dma_start(wstage[:], w[0:P, :])
            nc.sync.dma_start(wstage2[:], w[P:2 * P, :])
            for n in range(1, bp):
                emit_x_dma(n, 0)
                emit_x_dma(n, 1)

            # ---- signs (ACT) -------------------------------------------
            st = [const_pool.tile([P, KH * KW, O], BF16, name=f"st{ci}")
                  for ci in range(NCI)]
            for ci in range(NCI):
                nc.scalar.sign(st[ci][:], wpt[ci][:], bias=zbias[:])

            # ---- a = mean|w| + A'^T column scales, all on ACT -----------
            # av[c][:, co]: c=0 -> a, c=1 -> a/3, c=2 -> a/12  (fp32)
            av = [const_pool.tile([P, NCO], F32, name=f"av{c}")
                  for c in range(3)]
            wabs = wstage_pool.tile([P, KIN], BF16, name="wabs", bufs=1)
            for co, ws in ((0, wstage), (1, wstage2)):
                nc.scalar.activation(wabs[:], ws[:],
                                     mybir.ActivationFunctionType.Abs,
                                     scale=1.0 / KIN,
                                     accum_out=av[0][:, co:co + 1])
            for co in range(NCO):
                nc.scalar.mul(av[1][:, co:co + 1], av[0][:, co:co + 1],
                              1.0 / 3.0)
                nc.scalar.mul(av[2][:, co:co + 1], av[0][:, co:co + 1],
                              1.0 / 12.0)
            av_of_p = [av[0], av[1], av[1], av[2], av[2], av[0]]

            # ---- U = G'.sign(w): dyadic-exact bf16, built on DVE --------
            upack = const_pool.tile([P, NP * KH * NCI * O], BF16)
            up = upack[:].rearrange("q (p kh ci o) -> q p kh ci o",
                                    p=NP, kh=KH, ci=NCI)
            ust: list[dict] = [{}, {}]

            def _sv(ci):
                sv = st[ci][:].rearrange("q (kh kw) o -> q kh kw o", kh=KH)
                return sv[:, :, 0], sv[:, :, 1], sv[:, :, 2]

            def utmp():
                return tmp_pool.tile([P, KH, O], BF16, name="ut", bufs=8)

            def emit_u(ci, piece):
                s0, s1, s2 = _sv(ci)
                u = [up[:, p, :, ci] for p in range(NP)]
                t = ust[ci]
                vv = nc.vector
                if piece == 0:
                    vv.tensor_scalar_mul(u[0], s0, 0.25)
                elif piece == 1:
                    t['t1'], t['s1h'], t['t2'] = utmp(), utmp(), utmp()
                    vv.tensor_tensor(t['t1'][:], s0, s2, op=ALU.add)
                    vv.tensor_scalar_mul(t['s1h'][:], s1, 0.5)
                    vv.tensor_scalar_mul(t['t2'][:], t['t1'][:], -0.5)
                    vv.tensor_tensor(u[1], t['t2'][:], t['s1h'][:],
                                     op=ALU.subtract)
                elif piece == 2:
                    vv.tensor_tensor(u[2], u[1], s1, op=ALU.add)
                elif piece == 3:
                    t['q2t'], t['q'], t['s0h'] = utmp(), utmp(), utmp()
                    vv.tensor_scalar_mul(t['q2t'][:], s2, 2.0)
                    vv.tensor_tensor(t['q'][:], t['q2t'][:], s1, op=ALU.add)
                    vv.tensor_scalar_mul(t['s0h'][:], s0, 0.5)
                    vv.tensor_tensor(u[3], t['s0h'][:], t['q'][:],
                                     op=ALU.add)
                elif piece == 4:
                    t['s1d'] = utmp()
                    vv.tensor_scalar_mul(t['s1d'][:], s1, 2.0)
                    vv.tensor_tensor(u[4], u[3], t['s1d'][:],
                                     op=ALU.subtract)
                else:
                    vv.tensor_copy(u[5], s2)

            # ---- input transform: V[p] = B^T d (DVE bf16) ---------------
            def make_tstate(n, ci):
                xt = xts[n][ci]
                d = [xt[:, s, 0:FLAT] for s in range(NP)]
                vt = v_pool.tile([P, NP, FLAT], BF16, name="vt")
                return {'d': d, 'vt': vt, 't': {}}

            def emit_plane(s, p):
                d, vt, t = s['d'], s['vt'], s['t']
                v = vt[:, p, :]
                vv = nc.vector

                def tmp():
                    return tmp_pool.tile([P, FLAT], BF16, name="tw")

                if p == 0:
                    t['i'], t['k'], t['k4'] = tmp(), tmp(), tmp()
                    vv.tensor_tensor(t['i'][:], d[4], d[2], op=ALU.subtract)
                    vv.tensor_tensor(t['k'][:], d[0], d[2], op=ALU.subtract)
                    vv.tensor_scalar_mul(t['k4'][:], t['k'][:], 4.0)
                    vv.tensor_tensor(v, t['k4'][:], t['i'][:], op=ALU.add)
                elif p == 1:
                    t['ap'], t['bp'], t['a4'] = tmp(), tmp(), tmp()
                    vv.tensor_tensor(t['ap'][:], d[1], d[2], op=ALU.add)
                    vv.tensor_tensor(t['bp'][:], d[3], d[4], op=ALU.add)
                    vv.tensor_scalar_mul(t['a4'][:], t['ap'][:], -4.0)
                    vv.tensor_tensor(v, t['a4'][:], t['bp'][:], op=ALU.add)
                elif p == 2:
                    t['am'], t['f'], t['am4'] = tmp(), tmp(), tmp()
                    vv.tensor_tensor(t['am'][:], d[1], d[2], op=ALU.subtract)
                    vv.tensor_tensor(t['f'][:], d[3], d[4], op=ALU.subtract)
                    vv.tensor_scalar_mul(t['am4'][:], t['am'][:], 4.0)
                    vv.tensor_tensor(v, t['am4'][:], t['f'][:],
                                     op=ALU.subtract)
                elif p == 3:
                    t['g'], t['g2'] = tmp(), tmp()
                    vv.tensor_tensor(t['g'][:], d[3], d[1], op=ALU.subtract)
                    vv.tensor_scalar_mul(t['g2'][:], t['g'][:], 2.0)
                    vv.tensor_tensor(v, t['g2'][:], t['i'][:], op=ALU.add)
                elif p == 4:
                    vv.tensor_tensor(v, t['i'][:], t['g2'][:],
                                     op=ALU.subtract)
                else:
                    t['nn'], t['g4'] = tmp(), tmp()
                    vv.tensor_tensor(t['nn'][:], d[5], d[3], op=ALU.subtract)
                    vv.tensor_scalar_mul(t['g4'][:], t['g'][:], -4.0)
                    vv.tensor_tensor(v, t['g4'][:], t['nn'][:], op=ALU.add)

            def emit_transform(n, ci):
                s = make_tstate(n, ci)
                for p in range(NP):
                    emit_plane(s, p)
                return s['vt']

            def emit_img0_transforms():
                """Image 0: planes emitted p-major across both ci chunks,
                with the U-build pieces woven in, so the PE's p-th matmul
                group unblocks as early as possible."""
                emit_u(0, 0)
                emit_u(1, 0)
                ss = [make_tstate(0, 0), make_tstate(0, 1)]
                for p in range(NP):
                    for ci in range(NCI):
                        emit_plane(ss[ci], p)
                    if p < NP - 1:
                        emit_u(0, p + 1)
                        emit_u(1, p + 1)
                return [ss[0]['vt'], ss[1]['vt']]

            # ---- matmuls + eviction -------------------------------------
            def emit_mms_co(n, vts, co):
                """Matmuls for one co chunk of image n + ACT evictions.

                Each (co,p) accumulates into a 2-bank PSUM pair-tile
                [P,1024] (batch windows at 0 and 512, each within a bank);
                one ACT eviction drains both batches, scaled by a*c_p, into
                a contiguous bf16 E plane [P, 840].  The two MMs sharing a
                stationary U tile are adjacent (halves exposed LDWEIGHTS).
                """
                ev = [None] * NP
                for p in range(NP):
                    ps = psum_pool.tile([P, 1024], F32, name="ps")
                    psv = ps[:].rearrange("q (b g) -> q b g", b=2)
                    for ci in range(NCI):
                        for kh in range(KH):
                            off = _u_off(p, kh, ci, co)
                            first = ci == 0 and kh == 0
                            last = ci == NCI - 1 and kh == KH - 1
                            for b2 in range(2):
                                rhs = vts[ci][:, p,
                                              PW15 * kh + FB * b2:
                                              PW15 * kh + FB * b2 + FB]
                                nc.tensor.matmul(
                                    ps[:, 512 * b2:512 * b2 + FB],
                                    upack[:, off:off + P],
                                    rhs, start=first, stop=last,
                                )
                    et = e_pool.tile([P, OUTF], BF16, name="et")
                    nc.scalar.mul(et[:].rearrange("q (b f) -> q b f", b=2),
                                  psv[:, :, 0:FB],
                                  av_of_p[p][:, co:co + 1])
                    ev[p] = et
                return ev

            def emit_y_co(n, co, e):
                """Inverse transform A'^T: tensor_tensor on DVE (bf16 2x),
                unary scales on ACT, both batches fused (contiguous 840).
                Output DMA triggers ride the sync ring."""

                def yt():
                    return y_pool.tile([P, OUTF], BF16, name="yw")

                s_, d_, pp, q_ = yt(), yt(), yt(), yt()
                o1, q2, p4, q8, o2 = yt(), yt(), yt(), yt(), yt()
                y = [yt() for _ in range(4)]
                vv = nc.vector
                sc = nc.scalar
                # pure-DVE E-consuming ops first: every E plane except E5 is
                # released before any ACT-dependent op, so ACT's FIFO (next
                # image's evictions queue ahead of this round's muls) can
                # always recycle e_pool buffers -> no cross-engine cycle.
                vv.tensor_tensor(s_[:], e[1][:], e[2][:], op=ALU.add)
                vv.tensor_tensor(pp[:], e[3][:], e[4][:], op=ALU.add)
                vv.tensor_tensor(d_[:], e[1][:], e[2][:], op=ALU.subtract)
                vv.tensor_tensor(q_[:], e[3][:], e[4][:], op=ALU.subtract)
                vv.tensor_tensor(o1[:], s_[:], e[0][:], op=ALU.add)
                vv.tensor_tensor(y[0][:], pp[:], o1[:], op=ALU.add)
                sc.mul(q8[:], q_[:], 8.0)
                sc.mul(q2[:], q_[:], 2.0)
                sc.mul(p4[:], pp[:], 4.0)
                vv.tensor_tensor(o2[:], q8[:], d_[:], op=ALU.add)
                vv.tensor_tensor(y[3][:], o2[:], e[5][:], op=ALU.add)
                vv.tensor_tensor(y[1][:], q2[:], d_[:], op=ALU.add)
                vv.tensor_tensor(y[2][:], p4[:], s_[:], op=ALU.add)
                # scalar ring: decoupled from the input-DMA sync ring, so a
                # held y tile can never transitively wait on a future
                # transform (sync-ring x DMAs block on xph buffer recycling)
                for u_ in range(4):
                    nc.scalar.dma_start(
                        out[n, co * P:(co + 1) * P, u_, :], y[u_][:])

            # ---- software-pipelined emission ----------------------------
            vts = emit_img0_transforms()
            evs: list = [None] * bp
            for n in range(bp):
                if n >= 1:
                    for co in range(NCO):
                        emit_y_co(n - 1, co, evs[n - 1][co])
                if n == bp - 1:
                    # last image: overlap co0's inverse transform with
                    # co1's matmuls to shrink the tail
                    ev0 = emit_mms_co(n, vts, 0)
                    emit_y_co(n, 0, ev0)
                    ev1 = emit_mms_co(n, vts, 1)
                    emit_y_co(n, 1, ev1)
                else:
                    evs[n] = [emit_mms_co(n, vts, co) for co in range(NCO)]
                    vts = [emit_transform(n + 1, 0),
                           emit_transform(n + 1, 1)]

    nc.compile()
    return nc


_NC_CACHE: dict[int, object] = {}


def _get_nc(bp: int = BP):
    if bp not in _NC_CACHE:
        _NC_CACHE[bp] = build(bp)
    return _NC_CACHE[bp]


def make_in_maps(x: np.ndarray, weight: np.ndarray, n_cores: int = N_CORES,
                 bp: int = BP):
    x = np.ascontiguousarray(x, dtype=np.float32)
    weight = np.ascontiguousarray(weight, dtype=np.float32)
    # d-plane marshalling (layout only): padded cols j=w+1 in 0..61,
    # d_s[r, t] = padded[r, 4t+s], flattened [58*15], stride-872 planes.
    padded = np.zeros((B, C, PR, 62), np.float32)
    padded[:, :, 1:H + 1, 1:W + 1] = x
    padded = padded.astype(ml_dtypes.bfloat16)
    xm = np.zeros((B, C, NP, DPL), ml_dtypes.bfloat16)
    for s in range(NP):
        xm[:, :, s, :FLAT] = padded[:, :, :, s::4][:, :, :, :PW15].reshape(
            B, C, FLAT)
    wq = weight.reshape(O, KIN).astype(ml_dtypes.bfloat16)
    wp = np.ascontiguousarray(
        weight.reshape(O, C, KH * KW).transpose(2, 1, 0)
    ).astype(ml_dtypes.bfloat16)  # [t, i, o]
    return [
        {"x": xm[i * bp:(i + 1) * bp], "w": wq, "wp": wp}
        for i in range(n_cores)
    ]


def kernel(x: np.ndarray, weight: np.ndarray) -> np.ndarray:
    nc = _get_nc(BP)
    in_maps = make_in_maps(x, weight)
    res = run_bass_kernel_spmd(nc, in_maps, core_ids=list(range(N_CORES)))
    out = np.empty((B, O, H, W), dtype=np.float32)
    for i in range(N_CORES):
        od = res.results[i]["out"]  # [bp, O, 4, 840] bf16
        od = od.astype(np.float32).reshape(BP, O, 4, H, PW15)
        od = od.transpose(0, 1, 3, 4, 2).reshape(BP, O, H, PW15 * 4)
        out[i * BP:(i + 1) * BP] = od[:, :, :, :W]
    return out


# revision 27
# speedup vs baseline: 1.0678x; 1.0678x over previous
"""BinaryConv (XNOR-style binary-weight 3x3 conv) on 8 Trainium2 NeuronCores.

Full-input contract: kernel(x=[32,256,56,56] f32, weight=[256,256,3,3] f32)
-> [32,256,56,56] f32.

Strategy: data-parallel over batch (4 images/core), weight replicated.
Per core, a 1-D Winograd F(4,3) decomposition along W cuts the tensor-engine
MAC count 2x vs direct convolution (6 transformed positions per 4 output
columns instead of 12 tap-MACs):

  y = A'^T [ (G' s) . (B^T d) ]   per output row, with the 3 kh taps and the
                                  2 ci chunks accumulated in PSUM.

All math on device; host marshalling is layout/dtype only: x ships bf16,
de-interleaved into the six B^T operand planes d0..d5 (stride-4 phases of the
zero-padded rows, flattened 58x15 with a garbage 15th column) so every DVE op
and matmul rhs reads a contiguous, 4B-aligned window.  The weight transform
G'·sign(w) uses the row-rescaled dyadic G' (rows x[1,3,3,12,12,1]) so U is
exact in bf16; the matching column scales 1/c_p fold into the per-channel
a=mean|w| (ACT |.|+accum passes) applied at PSUM eviction on ACT.  Each
(co,p) pair accumulates into a 2-bank PSUM pair-tile so one eviction drains
both output batches into a contiguous bf16 E plane.  The input transform B^T
and inverse transform A'^T run on DVE in bf16 tensor_tensor (2x mode) with
unary scales on DVE(4x)/ACT; image 0's transforms are interleaved across
both ci chunks plane-by-plane (with the U-build woven in) so the PE ramps
immediately, and the last image's inverse transform overlaps its second
co-chunk matmuls.  The output ships phase-major bf16, re-interleaved f32 on
host.
"""

import ml_dtypes
import numpy as np

import concourse.mybir as mybir
import concourse.tile as tile
from concourse import bacc
from concourse.bass_utils import run_bass_kernel_spmd

F32 = mybir.dt.float32
BF16 = mybir.dt.bfloat16
ALU = mybir.AluOpType

N_CORES = 8
B, C, H, W = 32, 256, 56, 56
O, KH, KW = 256, 3, 3
BP = B // N_CORES            # images per core
P = 128                      # partitions
NCI = C // P                 # input-channel chunks
NCO = O // P                 # output-channel chunks
NP = 6                       # winograd positions (F(4,3): m+r-1 = 6)
TX = 14                      # output tiles along W (4 cols each)
PR = H + 2                   # padded rows (h -1..56)
PW15 = 15                    # tile columns incl. garbage col 14
DPL = PR * PW15 + 2          # 872: d-plane stride (870 valid + 2 pad)
FLAT = PR * PW15             # 870: flat transform window
OUTF = H * PW15              # 840: output flat length per (co, img)
FB = OUTF // 2               # 420: psum free size (2 batches)
KIN = C * KH * KW            # 2304 per-filter fan-in


def _u_off(p: int, kh: int, ci: int, co: int) -> int:
    return (((p * KH + kh) * NCI + ci) * NCO + co) * P


def build(bp: int = BP):
    nc = bacc.Bacc(
        "TRN2",
        target_bir_lowering=False,
        debug=False,
        enable_asserts=False,
        num_devices=N_CORES,
        enable_partition_id=False,
    )
    x_d = nc.dram_tensor("x", [bp, C, NP, DPL], BF16, kind="ExternalInput")
    w_d = nc.dram_tensor("w", [O, KIN], BF16, kind="ExternalInput")
    wp_d = nc.dram_tensor("wp", [KH * KW, C, O], BF16, kind="ExternalInput")
    out_d = nc.dram_tensor("out", [bp, O, 4, OUTF], BF16,
                           kind="ExternalOutput")

    x = x_d.ap().rearrange("n (c p) s f -> n p c s f", p=P)
    w = w_d.ap()
    wp = wp_d.ap().rearrange("t (c p) o -> p c t o", p=P)
    out = out_d.ap()

    with tile.TileContext(nc) as tc:
        with (
            tc.tile_pool(name="const", bufs=1) as const_pool,
            tc.tile_pool(name="wstage", bufs=2) as wstage_pool,
            tc.tile_pool(name="xph", bufs=2) as xph_pool,
            tc.tile_pool(name="vt", bufs=4) as v_pool,
            tc.tile_pool(name="tmp", bufs=13) as tmp_pool,
            tc.tile_pool(name="ev", bufs=7) as e_pool,
            tc.tile_pool(name="yt", bufs=13) as y_pool,
            tc.tile_pool(name="psum", bufs=3, space="PSUM") as psum_pool,
            tc.tile_pool(name="warmps", bufs=1, space="PSUM") as warmps_pool,
        ):
            # ---- PE warmup: hold HAM at 2.4GHz through the input ramp ----
            warm_l = const_pool.tile([P, P], BF16)
            warm_r = const_pool.tile([P, 512], BF16)
            nc.gpsimd.memset(warm_l[:], 0.0)
            nc.gpsimd.memset(warm_r[:], 0.0)
            zbias = const_pool.tile([P, 1], F32)
            zscr = const_pool.tile([P, 1], F32)
            nc.gpsimd.memset(zbias[:], 0.0)
            warm_ps = warmps_pool.tile([P, 512], F32)
            for _ in range(14):
                nc.tensor.matmul(warm_ps[:], warm_l[:], warm_r[:],
                                 start=True, stop=True)
            # preload the Sign LUT on ACT before the weights arrive
            nc.scalar.sign(zscr[:], zbias[:], bias=zbias[:])

            # ---- critical-path input DMAs on the sync ring (FIFO) --------
            wpt = [const_pool.tile([P, KH * KW, O], BF16, name=f"wpt{ci}")
                   for ci in range(NCI)]
            wstage = wstage_pool.tile([P, KIN], BF16, name="ws")
            wstage2 = wstage_pool.tile([P, KIN], BF16, name="ws2")
            xts: list[list] = [[None] * NCI for _ in range(bp)]

            def emit_x_dma(n, ci):
                # two flat-AP halves: planes are host-ordered
                # [d2,d4,d0 | d1,d3,d5] so V0's operands land first
                xt = xph_pool.tile([P, NP, DPL], BF16, name="xt")
                for h in range(2):
                    nc.sync.dma_start(
                        xt[:, 3 * h:3 * h + 3, :].rearrange(
                            "q s f -> q (s f)"),
                        x[n, :, ci, 3 * h:3 * h + 3].rearrange(
                            "q s f -> q (s f)"),
                    )
                xts[n][ci] = xt

            nc.sync.dma_start(wpt[0][:], wp[:, 0])
            emit_x_dma(0, 0)
            emit_x_dma(0, 1)
            nc.sync.dma_start(wpt[1][:], wp[:, 1])
            nc.sync.dma_start(wstage[:], w[0:P, :])
            nc.sync.dma_start(wstage2[:], w[P:2 * P, :])
            for n in range(1, bp):
                emit_x_dma(n, 0)
                emit_x_dma(n, 1)

            # ---- signs (ACT) -------------------------------------------
            st = [const_pool.tile([P, KH * KW, O], BF16, name=f"st{ci}")
                  for ci in range(NCI)]
            for ci in range(NCI):
                nc.scalar.sign(st[ci][:], wpt[ci][:], bias=zbias[:])

            # ---- a = mean|w| + A'^T column scales, all on ACT -----------
            # av[c][:, co]: c=0 -> a, c=1 -> a/3, c=2 -> a/12  (fp32)
            av = [const_pool.tile([P, NCO], F32, name=f"av{c}")
                  for c in range(3)]
            wabs = wstage_pool.tile([P, KIN], BF16, name="wabs", bufs=1)
            for co, ws in ((0, wstage), (1, wstage2)):
                nc.scalar.activation(wabs[:], ws[:],
                                     mybir.ActivationFunctionType.Abs,
                                     scale=1.0 / KIN,
                                     accum_out=av[0][:, co:co + 1])
            for co in range(NCO):
                nc.scalar.mul(av[1][:, co:co + 1], av[0][:, co:co + 1],
                              1.0 / 3.0)
                nc.scalar.mul(av[2][:, co:co + 1], av[0][:, co:co + 1],
                              1.0 / 12.0)
            av_of_p = [av[0], av[1], av[1], av[2], av[2], av[0]]

            # ---- U = G'.sign(w): dyadic-exact bf16, built on DVE --------
            upack = const_pool.tile([P, NP * KH * NCI * O], BF16)
            up = upack[:].rearrange("q (p kh ci o) -> q p kh ci o",
                                    p=NP, kh=KH, ci=NCI)
            ust: list[dict] = [{}, {}]

            def _sv(ci):
                sv = st[ci][:].rearrange("q (kh kw) o -> q kh kw o", kh=KH)
                return sv[:, :, 0], sv[:, :, 1], sv[:, :, 2]

            def utmp():
                return tmp_pool.tile([P, KH, O], BF16, name="ut", bufs=8)

            def emit_u(ci, piece):
                s0, s1, s2 = _sv(ci)
                u = [up[:, p, :, ci] for p in range(NP)]
                t = ust[ci]
                vv = nc.vector
                if piece == 0:
                    vv.tensor_scalar_mul(u[0], s0, 0.25)
                elif piece == 1:
                    t['t1'], t['s1h'], t['t2'] = utmp(), utmp(), utmp()
                    vv.tensor_tensor(t['t1'][:], s0, s2, op=ALU.add)
                    vv.tensor_scalar_mul(t['s1h'][:], s1, 0.5)
                    vv.tensor_scalar_mul(t['t2'][:], t['t1'][:], -0.5)
                    vv.tensor_tensor(u[1], t['t2'][:], t['s1h'][:],
                                     op=ALU.subtract)
                elif piece == 2:
                    vv.tensor_tensor(u[2], u[1], s1, op=ALU.add)
                elif piece == 3:
                    t['q2t'], t['q'], t['s0h'] = utmp(), utmp(), utmp()
                    vv.tensor_scalar_mul(t['q2t'][:], s2, 2.0)
                    vv.tensor_tensor(t['q'][:], t['q2t'][:], s1, op=ALU.add)
                    vv.tensor_scalar_mul(t['s0h'][:], s0, 0.5)
                    vv.tensor_tensor(u[3], t['s0h'][:], t['q'][:],
                                     op=ALU.add)
                elif piece == 4:
                    t['s1d'] = utmp()
                    vv.tensor_scalar_mul(t['s1d'][:], s1, 2.0)
                    vv.tensor_tensor(u[4], u[3], t['s1d'][:],
                                     op=ALU.subtract)
                else:
                    vv.tensor_copy(u[5], s2)

            # ---- input transform: V[p] = B^T d (DVE bf16) ---------------
            # host plane order [d2,d4,d0,d1,d3,d5] -> logical d index
            DSLOT = {2: 0, 4: 1, 0: 2, 1: 3, 3: 4, 5: 5}

            def make_tstate(n, ci):
                xt = xts[n][ci]
                d = [xt[:, DSLOT[s], 0:FLAT] for s in range(NP)]
                vt = v_pool.tile([P, NP, FLAT], BF16, name="vt")
                return {'d': d, 'vt': vt, 't': {}}

            def emit_plane(s, p):
                d, vt, t = s['d'], s['vt'], s['t']
                v = vt[:, p, :]
                vv = nc.vector

                def tmp():
                    return tmp_pool.tile([P, FLAT], BF16, name="tw")

                if p == 0:
                    t['i'], t['k'], t['k4'] = tmp(), tmp(), tmp()
                    vv.tensor_tensor(t['i'][:], d[4], d[2], op=ALU.subtract)
                    vv.tensor_tensor(t['k'][:], d[0], d[2], op=ALU.subtract)
                    vv.tensor_scalar_mul(t['k4'][:], t['k'][:], 4.0)
                    vv.tensor_tensor(v, t['k4'][:], t['i'][:], op=ALU.add)
                elif p == 1:
                    # butterfly: r = d4-4*d2, s = d3-4*d1 -> V1 = r+s,
                    # V2 = r-s (saves two tensor_tensor vs direct rows)
                    t['t2'], t['r'] = tmp(), tmp()
                    t['t1'], t['s'] = tmp(), tmp()
                    vv.tensor_scalar_mul(t['t2'][:], d[2], 4.0)
                    vv.tensor_tensor(t['r'][:], d[4], t['t2'][:],
                                     op=ALU.subtract)
                    vv.tensor_scalar_mul(t['t1'][:], d[1], 4.0)
                    vv.tensor_tensor(t['s'][:], d[3], t['t1'][:],
                                     op=ALU.subtract)
                    vv.tensor_tensor(v, t['r'][:], t['s'][:], op=ALU.add)
                elif p == 2:
                    vv.tensor_tensor(v, t['r'][:], t['s'][:],
                                     op=ALU.subtract)
                elif p == 3:
                    t['g'], t['g2'] = tmp(), tmp()
                    vv.tensor_tensor(t['g'][:], d[3], d[1], op=ALU.subtract)
                    vv.tensor_scalar_mul(t['g2'][:], t['g'][:], 2.0)
                    vv.tensor_tensor(v, t['g2'][:], t['i'][:], op=ALU.add)
                elif p == 4:
                    vv.tensor_tensor(v, t['i'][:], t['g2'][:],
                                     op=ALU.subtract)
                else:
                    t['nn'], t['g4'] = tmp(), tmp()
                    vv.tensor_tensor(t['nn'][:], d[5], d[3], op=ALU.subtract)
                    vv.tensor_scalar_mul(t['g4'][:], t['g'][:], -4.0)
                    vv.tensor_tensor(v, t['g4'][:], t['nn'][:], op=ALU.add)

            def emit_transform(n, ci):
                s = make_tstate(n, ci)
                for p in range(NP):
                    emit_plane(s, p)
                return s['vt']

            def emit_img0_transforms():
                """Image 0: planes emitted p-major across both ci chunks,
                with the U-build pieces woven in, so the PE's p-th matmul
                group unblocks as early as possible."""
                emit_u(0, 0)
                emit_u(1, 0)
                ss = [make_tstate(0, 0), make_tstate(0, 1)]
                for p in range(NP):
                    for ci in range(NCI):
                        emit_plane(ss[ci], p)
                    if p < NP - 1:
                        emit_u(0, p + 1)
                        emit_u(1, p + 1)
                return [ss[0]['vt'], ss[1]['vt']]

            # ---- matmuls + eviction -------------------------------------
            def emit_mms_co(n, vts, co):
                """Matmuls for one co chunk of image n + ACT evictions.

                Each (co,p) accumulates into a 2-bank PSUM pair-tile
                [P,1024] (batch windows at 0 and 512, each within a bank);
                one ACT eviction drains both batches, scaled by a*c_p, into
                a contiguous bf16 E plane [P, 840].  The two MMs sharing a
                stationary U tile are adjacent (halves exposed LDWEIGHTS).
                """
                ev = [None] * NP
                for p in range(NP):
                    ps = psum_pool.tile([P, 1024], F32, name="ps")
                    psv = ps[:].rearrange("q (b g) -> q b g", b=2)
                    for ci in range(NCI):
                        for kh in range(KH):
                            off = _u_off(p, kh, ci, co)
                            first = ci == 0 and kh == 0
                            last = ci == NCI - 1 and kh == KH - 1
                            for b2 in range(2):
                                rhs = vts[ci][:, p,
                                              PW15 * kh + FB * b2:
                                              PW15 * kh + FB * b2 + FB]
                                nc.tensor.matmul(
                                    ps[:, 512 * b2:512 * b2 + FB],
                                    upack[:, off:off + P],
                                    rhs, start=first, stop=last,
                                )
                    et = e_pool.tile([P, OUTF], BF16, name="et")
                    nc.scalar.mul(et[:].rearrange("q (b f) -> q b f", b=2),
                                  psv[:, :, 0:FB],
                                  av_of_p[p][:, co:co + 1])
                    ev[p] = et
                return ev

            def emit_y_co(n, co, e):
                """Inverse transform A'^T: tensor_tensor on DVE (bf16 2x),
                unary scales on ACT, both batches fused (contiguous 840).
                Output DMA triggers ride the sync ring."""

                def yt():
                    return y_pool.tile([P, OUTF], BF16, name="yw")

                s_, d_, pp, q_ = yt(), yt(), yt(), yt()
                o1, q2, p4, q8, o2 = yt(), yt(), yt(), yt(), yt()
                y = [yt() for _ in range(4)]
                vv = nc.vector
                sc = nc.scalar
                # pure-DVE E-consuming ops first: every E plane except E5 is
                # released before any ACT-dependent op, so ACT's FIFO (next
                # image's evictions queue ahead of this round's muls) can
                # always recycle e_pool buffers -> no cross-engine cycle.
                vv.tensor_tensor(s_[:], e[1][:], e[2][:], op=ALU.add)
                vv.tensor_tensor(pp[:], e[3][:], e[4][:], op=ALU.add)
                vv.tensor_tensor(d_[:], e[1][:], e[2][:], op=ALU.subtract)
                vv.tensor_tensor(q_[:], e[3][:], e[4][:], op=ALU.subtract)
                vv.tensor_tensor(o1[:], s_[:], e[0][:], op=ALU.add)
                vv.tensor_tensor(y[0][:], pp[:], o1[:], op=ALU.add)
                sc.mul(q8[:], q_[:], 8.0)
                sc.mul(q2[:], q_[:], 2.0)
                sc.mul(p4[:], pp[:], 4.0)
                vv.tensor_tensor(o2[:], q8[:], d_[:], op=ALU.add)
                vv.tensor_tensor(y[3][:], o2[:], e[5][:], op=ALU.add)
                vv.tensor_tensor(y[1][:], q2[:], d_[:], op=ALU.add)
                vv.tensor_tensor(y[2][:], p4[:], s_[:], op=ALU.add)
                # scalar ring: decoupled from the input-DMA sync ring, so a
                # held y tile can never transitively wait on a future
                # transform (sync-ring x DMAs block on xph buffer recycling)
                for u_ in range(4):
                    nc.scalar.dma_start(
                        out[n, co * P:(co + 1) * P, u_, :], y[u_][:])

            # ---- software-pipelined emission ----------------------------
            vts = emit_img0_transforms()
            evs: list = [None] * bp
            for n in range(bp):
                if n >= 1:
                    for co in range(NCO):
                        emit_y_co(n - 1, co, evs[n - 1][co])
                if n == bp - 1:
                    # last image: overlap co0's inverse transform with
                    # co1's matmuls to shrink the tail
                    ev0 = emit_mms_co(n, vts, 0)
                    emit_y_co(n, 0, ev0)
                    ev1 = emit_mms_co(n, vts, 1)
                    emit_y_co(n, 1, ev1)
                else:
                    evs[n] = [emit_mms_co(n, vts, co) for co in range(NCO)]
                    vts = [emit_transform(n + 1, 0),
                           emit_transform(n + 1, 1)]

    nc.compile()
    return nc


_NC_CACHE: dict[int, object] = {}


def _get_nc(bp: int = BP):
    if bp not in _NC_CACHE:
        _NC_CACHE[bp] = build(bp)
    return _NC_CACHE[bp]


def make_in_maps(x: np.ndarray, weight: np.ndarray, n_cores: int = N_CORES,
                 bp: int = BP):
    x = np.ascontiguousarray(x, dtype=np.float32)
    weight = np.ascontiguousarray(weight, dtype=np.float32)
    # d-plane marshalling (layout only): padded cols j=w+1 in 0..61,
    # d_s[r, t] = padded[r, 4t+s], flattened [58*15], stride-872 planes.
    padded = np.zeros((B, C, PR, 62), np.float32)
    padded[:, :, 1:H + 1, 1:W + 1] = x
    padded = padded.astype(ml_dtypes.bfloat16)
    xm = np.zeros((B, C, NP, DPL), ml_dtypes.bfloat16)
    for si, s in enumerate((2, 4, 0, 1, 3, 5)):  # V0 operands first
        xm[:, :, si, :FLAT] = padded[:, :, :, s::4][:, :, :, :PW15].reshape(
            B, C, FLAT)
    wq = weight.reshape(O, KIN).astype(ml_dtypes.bfloat16)
    wp = np.ascontiguousarray(
        weight.reshape(O, C, KH * KW).transpose(2, 1, 0)
    ).astype(ml_dtypes.bfloat16)  # [t, i, o]
    return [
        {"x": xm[i * bp:(i + 1) * bp], "w": wq, "wp": wp}
        for i in range(n_cores)
    ]


def kernel(x: np.ndarray, weight: np.ndarray) -> np.ndarray:
    nc = _get_nc(BP)
    in_maps = make_in_maps(x, weight)
    res = run_bass_kernel_spmd(nc, in_maps, core_ids=list(range(N_CORES)))
    out = np.empty((B, O, H, W), dtype=np.float32)
    for i in range(N_CORES):
        od = res.results[i]["out"]  # [bp, O, 4, 840] bf16
        od = od.astype(np.float32).reshape(BP, O, 4, H, PW15)
        od = od.transpose(0, 1, 3, 4, 2).reshape(BP, O, H, PW15 * 4)
        out[i * BP:(i + 1) * BP] = od[:, :, :, :W]
    return out
